# revision 45
# baseline (speedup 1.0000x reference)
"""CharLSTM Trainium2 kernel.

Model (see problem): tokens [512, 512] -> emb gather -> xw = x @ W_ih.T + biases
-> 512-step LSTM recurrence -> h_last @ W_cls.T + b_cls -> [512, 256] logits.

Strategy:
  * Data-parallel over batch: 8 cores x 64 sequences each.  Weights replicated.
  * Per core, state kept transposed: hT/cT = [128 (hid), 64 (batch)] split into
    S=2 pipelined sub-blocks of 32 so PE/ACT/DVE overlap across the serial
    T=512 chain.
  * The embedding + input projection + biases are fused into one 256x512 table
    (table = emb @ W_ih_r.T + b) computed on device once, kept in SBUF, and
    gathered per 8-step chunk with gpsimd dma_gather(transpose=True), which
    lands xwT directly in the [gate-dims, tokens] orientation the recurrence
    needs.  This avoids materializing the 64 MB xw tensor in HBM.
  * Gate columns are reordered to [i, f, o, g] and ALL four gates go through
    ONE Tanh activation: sigmoid(x) = (tanh(x/2)+1)/2, with the /2 folded
    into the weights and the states stored as m = 2c, h' = 2h (compensated in
    W_hh / W_cls on the host).  The whole c/h update is then 4 fused
    scalar_tensor_tensor ops on the DVE — no separate gate fixups.
  * xw is added into the PSUM gate pre-activations by an identity matmul
    (accumulating), keeping the add off the DVE.
"""

import os
import sys
from contextlib import ExitStack

import numpy as np

for _p in ("/opt/trn_rl_repo", "/opt/pypackages"):
    if _p not in sys.path and os.path.isdir(_p):
        sys.path.append(_p)

VOCAB, EMB, HID = 256, 32, 128
B, T = 512, 512
N_CORES = 8
BC = B // N_CORES  # 64 sequences per core
S = 2              # pipelined sub-blocks per core (sweep)
BS = BC // S       # 32
G4 = 4 * HID       # 512 gate dims


def build_kernel(t_steps=T, ch=8, debug=False, repeat=1):
    """Build + compile the per-core SPMD program. Returns the Bacc object."""
    import concourse.bacc as bacc
    import concourse.bass as bass
    import concourse.mybir as mybir
    import concourse.tile as tile

    dt = mybir.dt
    AF = mybir.ActivationFunctionType
    Alu = mybir.AluOpType
    f32, f16, i16 = dt.float32, dt.float16, dt.int16

    assert t_steps % ch == 0
    nidx_ch = ch * BC          # gathered tokens per chunk
    assert nidx_ch % 128 == 0

    nc = bacc.Bacc(
        "TRN2",
        target_bir_lowering=False,
        debug=debug,
        num_devices=N_CORES,
    )

    # ---- I/O ----
    embT_d = nc.dram_tensor("embT", [EMB + 1, VOCAB], f32, kind="ExternalInput")
    wih_d = nc.dram_tensor("wih", [EMB + 1, G4], f32, kind="ExternalInput")
    whh_d = nc.dram_tensor("whh", [HID, G4], f16, kind="ExternalInput")
    id_d = nc.dram_tensor("ident", [HID, HID], f16, kind="ExternalInput")
    wcls_d = nc.dram_tensor("wcls", [HID, VOCAB], f16, kind="ExternalInput")
    bcls_d = nc.dram_tensor("bcls", [1, VOCAB], f16, kind="ExternalInput")
    idxs_d = nc.dram_tensor(
        "idxs", [128, t_steps * BC // 16], i16, kind="ExternalInput"
    )
    out_d = nc.dram_tensor("out", [BC, VOCAB], f32, kind="ExternalOutput")

    with tile.TileContext(nc) as tc, ExitStack() as ctx:
        const = ctx.enter_context(tc.tile_pool(name="const", bufs=1))
        ptab = ctx.enter_context(
            tc.tile_pool(name="ptab", bufs=1, space=bass.MemorySpace.PSUM)
        )
        psg = ctx.enter_context(
            tc.tile_pool(name="psg", bufs=3 if S <= 2 else 6, space=bass.MemorySpace.PSUM)
        )
        gpool = ctx.enter_context(tc.tile_pool(name="gpool", bufs=3))
        spool = ctx.enter_context(tc.tile_pool(name="spool", bufs=8))

        # ---- load constants ----
        embT_sb = const.tile([EMB + 1, VOCAB], f32, tag="embT")
        nc.sync.dma_start(embT_sb[:], embT_d[:])
        wih_sb = const.tile([EMB + 1, G4], f32, tag="wih")
        nc.sync.dma_start(wih_sb[:], wih_d[:])
        whh_sb = const.tile([HID, G4], f16, tag="whh")
        nc.sync.dma_start(whh_sb[:], whh_d[:])
        id_sb = const.tile([HID, HID], f16, tag="ident")
        nc.sync.dma_start(id_sb[:], id_d[:])
        wcls_sb = const.tile([HID, VOCAB], f16, tag="wcls")
        nc.sync.dma_start(wcls_sb[:], wcls_d[:])
        bcls_sb = const.tile([1, VOCAB], f16, tag="bcls")
        nc.sync.dma_start(bcls_sb[:], bcls_d[:])
        idx_sb = const.tile([128, t_steps * BC // 16], i16, tag="idxs")
        nc.sync.dma_start(idx_sb[:], idxs_d[:])

        # ---- build the fused token table in SBUF ----
        # table row (vocab v) holds [i | f | o | 2*g] pre-activations incl. both
        # biases.  SBUF layout is rank-packed for dma_gather's SBUF-source mode:
        # partition p, sub-row r (of 2) = vocab row 2p + r; sub-row r occupies
        # fp16 columns [r*512, (r+1)*512).  The host permutes embT's columns so
        # chunk r's matmul directly produces the vocab rows {2j + r}.
        table = const.tile([128, 2 * G4], f16, tag="table")
        for r in range(2):
            pt = ptab.tile([128, G4], f32, tag="ptab")
            nc.tensor.matmul(
                pt[:],
                embT_sb[:, r * 128 : (r + 1) * 128],
                wih_sb[:],
                start=True,
                stop=True,
            )
            nc.vector.tensor_copy(table[:, r * G4 : (r + 1) * G4], pt[:])

        ones_sb = const.tile([1, BC], f16, tag="ones")
        nc.vector.memset(ones_sb[:], 1.0)

        # ---- state ----
        hT = []
        cT = []
        for s in range(S):
            h = const.tile([HID, BS], f16, tag=f"h{s}")
            c = const.tile([HID, BS], f32, tag=f"c{s}")
            nc.vector.memset(h[:], 0.0)
            nc.vector.memset(c[:], 0.0)
            hT.append(h)
            cT.append(c)

        # ---- recurrence ----
        n_chunks = t_steps // ch
        xw_tiles = {}

        def emit_gather(c):
            xw = gpool.tile([128, 4, nidx_ch], f16, tag="xw")
            nc.gpsimd.dma_gather(
                xw[:],
                table[:],
                idx_sb[:, c * (nidx_ch // 16) : (c + 1) * (nidx_ch // 16)],
                nidx_ch,
                nidx_ch,
                G4,
                transpose=True,
                single_packet=False,
                sbuf_tokens_per_rank=128,
                sbuf_free_dim_per_rank=G4 * 2,  # 1024 B: one full row per rank stripe
            )
            xw_tiles[c] = xw

        for rep in range(repeat):
          for c in range(n_chunks):
            if rep == 0 and c == 0:
                emit_gather(0)
            if c + 1 < n_chunks:
                emit_gather(c + 1)
            elif rep + 1 < repeat:
                emit_gather(0)
            xw = xw_tiles.pop(c)
            for k in range(ch):
                toff = k * BC
                for s in range(S):
                    sl = slice(toff + s * BS, toff + (s + 1) * BS)
                    ps = psg.tile([128, 4 * BS], f32, tag="ps" if S > 2 else f"ps{s}")
                    # xw contribution first: it does not depend on h, so PE
                    # runs it while waiting for h.  start=True zeroes the
                    # whole 2 KB PSUM bank granule; the gate matmuls then
                    # accumulate on top.
                    nc.tensor.matmul(
                        ps[:],
                        id_sb[:],
                        xw[:, :, sl],
                        start=True,
                        stop=False,
                        skip_group_check=True,
                    )
                    for gb in range(4):
                        nc.tensor.matmul(
                            ps[:, gb * BS : (gb + 1) * BS],
                            whh_sb[:, gb * HID : (gb + 1) * HID],
                            hT[s][:],
                            start=False,
                            stop=(gb == 3),
                            skip_group_check=True,
                        )
                    # All four gates through ONE Tanh: weights are host-scaled
                    # so sg = [ti, tf, to, g] with tx = tanh(zx/2) = 2*sig(zx)-1
                    # and g = tanh(zg).  State is m = 2c and hT = 2h (the 2x
                    # factors are folded into W_hh / W_cls on the host):
                    #   u2 = (ti+1)*g   = 2*i*g
                    #   u1 = (tf+1)*m   = 2*f*m
                    #   m' = 0.5*u1+u2  = f*m + 2*i*g = 2c'
                    #   tau = tanh(0.5*m) = tanh(c)
                    #   h' = (to+1)*tau = 2*o*tanh(c)
                    sg = spool.tile([128, 4 * BS], f16, tag=f"sg{s}")
                    nc.scalar.activation(sg[:], ps[:], AF.Tanh)
                    u2 = spool.tile([128, BS], f16, tag=f"u2{s}")
                    nc.vector.scalar_tensor_tensor(
                        u2[:], sg[:, 0:BS], 1.0, sg[:, 3 * BS : 4 * BS],
                        Alu.add, Alu.mult,
                    )
                    u1 = spool.tile([128, BS], f32, tag=f"u1{s}")
                    nc.vector.scalar_tensor_tensor(
                        u1[:], sg[:, BS : 2 * BS], 1.0, cT[s][:],
                        Alu.add, Alu.mult,
                    )
                    nc.vector.scalar_tensor_tensor(
                        cT[s][:], u1[:], 0.5, u2[:], Alu.mult, Alu.add
                    )
                    tau = spool.tile([128, BS], f16, tag=f"tau{s}")
                    nc.scalar.activation(tau[:], cT[s][:], AF.Tanh, scale=0.5)
                    nc.vector.scalar_tensor_tensor(
                        hT[s][:], sg[:, 2 * BS : 3 * BS], 1.0, tau[:],
                        Alu.add, Alu.mult,
                    )

        # ---- classifier ----
        hall = spool.tile([HID, BC], f16, tag="hall")
        for s in range(S):
            nc.vector.tensor_copy(hall[:, s * BS : (s + 1) * BS], hT[s][:])
        pc = ptab.tile([BC, VOCAB], f32, tag="pcls")
        nc.tensor.matmul(
            pc[:],
            hall[:],
            wcls_sb[:],
            start=True,
            stop=False,
            skip_group_check=True,
        )
        nc.tensor.matmul(
            pc[:],
            ones_sb[:],
            bcls_sb[:],
            start=False,
            stop=True,
            skip_group_check=True,
        )
        out_sb = spool.tile([BC, VOCAB], f32, tag="out")
        nc.vector.tensor_copy(out_sb[:], pc[:])
        nc.sync.dma_start(out_d[:], out_sb[:])

    nc.compile()
    return nc


def prep_inputs(inputs, emb, W_ih, W_hh, b_ih, b_hh, W_cls, b_cls, t_steps=T):
    """Host-side input marshaling: gate reorder [i,f,o,g], g-gate 2x pre-scale,
    transposes, vocab interleave permutation, and per-core token index wrap."""
    perm = np.concatenate(
        [np.arange(0, 128), np.arange(128, 256), np.arange(384, 512),
         np.arange(256, 384)]
    )
    Wih_r = np.asarray(W_ih, np.float32)[perm].copy()
    Whh_r = np.asarray(W_hh, np.float32)[perm].copy()
    bias_r = (np.asarray(b_ih, np.float32) + np.asarray(b_hh, np.float32))[perm].copy()
    # tanh parameterization: i,f,o pre-activations halved (sig(x) =
    # (tanh(x/2)+1)/2); g unscaled (tanh direct).  The recurrent/classifier
    # weights get an extra 0.5 because the stored hidden state is h' = 2h.
    Wih_r[: 3 * HID] *= 0.5
    bias_r[: 3 * HID] *= 0.5
    Whh_r[: 3 * HID] *= 0.25
    Whh_r[3 * HID :] *= 0.5

    embT_perm = np.concatenate(
        [np.asarray(emb, np.float32).T, np.ones((1, VOCAB), np.float32)], axis=0
    )  # [33, 256]; chunk r cols = vocab [128r, 128r+128)
    wih_aug = np.concatenate([Wih_r.T, bias_r[None, :]], axis=0)  # [33, 512]
    wih_aug = np.ascontiguousarray(wih_aug)

    common = {
        "embT": embT_perm.astype(np.float32),
        "wih": wih_aug.astype(np.float32),
        "whh": np.ascontiguousarray(Whh_r.T).astype(np.float16),
        "ident": np.eye(HID, dtype=np.float16),
        "wcls": np.ascontiguousarray(0.5 * np.asarray(W_cls, np.float32).T).astype(
            np.float16
        ),
        "bcls": np.asarray(b_cls, np.float32)[None, :].astype(np.float16),
    }

    tok = np.asarray(inputs)
    in_maps = []
    for cidx in range(N_CORES):
        tc_ = tok[cidx * BC : (cidx + 1) * BC, :t_steps]  # [64, t]
        flat = tc_.T.reshape(-1).astype(np.int16)  # t-major: idx j = t*64 + b
        wrapped = flat.reshape(-1, 16).T  # [16, n/16]; idx j at [j%16, j//16]
        idxs = np.ascontiguousarray(np.tile(wrapped, (8, 1)))  # [128, n/16]
        m = dict(common)
        m["idxs"] = idxs
        in_maps.append(m)
    return in_maps


_NC_CACHE = {}


def kernel(inputs, emb, W_ih, W_hh, b_ih, b_hh, W_cls, b_cls):
    import concourse.bass_utils as bass_utils

    if "nc" not in _NC_CACHE:
        _NC_CACHE["nc"] = build_kernel()
    nc = _NC_CACHE["nc"]
    in_maps = prep_inputs(inputs, emb, W_ih, W_hh, b_ih, b_hh, W_cls, b_cls)
    res = bass_utils.run_bass_kernel_spmd(
        nc, in_maps, core_ids=list(range(N_CORES))
    )
    out = np.concatenate([r["out"] for r in res.results], axis=0)
    return np.ascontiguousarray(out.astype(np.float32))
